# revision 16
# baseline (speedup 1.0000x reference)
"""Distributed GQA attention kernel for 8 TRN2 NeuronCores.

Problem: B=2, S=2048, D=2048, 32 q-heads / 8 kv-heads, hd=64, causal + RoPE.

Strategy (sequence-sharded "context parallel", single SPMD program):
  - Each core owns 2 zigzag row-blocks per batch (blocks bj=15-i and bi=i of
    16, stored [bj | bi]), 512 rows total. It computes Q for all 32 heads on
    its rows, K/V for all 8 kv-heads on its rows, applies RoPE, then
    AllGathers K/V in block-major layout (~1MB/rank).
  - Attention runs fully "transposed": projections produce qT/kT (head-dim on
    partitions) directly from x^T (host-pretransposed), scoresT = kT_tile.T @
    qT come out with keys on partitions, probsT feeds P@V as the moving
    operand with V as the stationary operand, and the PV output outT
    [hd, rows] is exactly the lhsT layout the output projection needs.
  - Uniform causal-skip width profile: q cols per batch are laid out
    [h0:HI | h1:HI | h0:LO | h1:LO] (HI = bj block rows, LO = bi rows).
    Key blocks kb=0..7 run 512-wide (every core needs its LO rows there and
    all HI rows attend them unmasked); kb=8..15 run 256-wide (HI only).
    This covers every core's causal needs with one instruction stream; the
    per-core mask *data* (multiplicative exp(mask) factors) zeroes the
    overcomputed region. 6144 score-cycles/combo vs 8192 unskipped.
  - Scores for the (a,p0)/(a,p1) head pairs land in one 2-bank PSUM tile
    [128,1024] (likewise the b pairs) so one ACT instruction exps both.
  - Softmax without max-subtraction: probs = exp(s/8). Full slots multiply
    masks only into the LO half (HI is always below-diagonal there); half
    slots multiply the HI cols. A-group muls run on Vector, B-group on
    GpSimd. The denominator comes free from a ones-column appended to V
    (M=65 PV matmuls); normalization multiplies the attention output.
  - Matmuls run in bf16 (1 cycle/row); psums/softmax stay fp32.

kernel(**inputs) -> np.ndarray  takes full inputs, returns full [2,2048,2048].
"""

import functools
import os
import sys
import types

import numpy as np
import ml_dtypes


BF16 = ml_dtypes.bfloat16

B, S, D = 2, 2048, 2048
NH, NKV, HD = 32, 8, 64
NREP = NH // NKV
NCORES = 8
BLK = 128
NBLK = S // BLK          # 16 blocks per batch
RPB = 2 * BLK            # rows per core per batch (2 blocks)
RT = B * RPB             # rows per core total = 512
KD = NKV * HD            # 512
VROW = 2 * HD + 2        # 130: [v_a | 1 | v_b | 1] per kv pair


def _heads_of_tile(t):
    gg, m = divmod(t, 4)
    return 8 * gg + m, 8 * gg + 4 + m


def _core_blocks(i):
    return i, NBLK - 1 - i


# --------------------------------------------------------------------------
# device graph
# --------------------------------------------------------------------------

@functools.lru_cache(maxsize=None)
def _build_nc():
    import concourse.bacc as bacc
    import concourse.mybir as mybir
    import concourse.tile as tile

    BF = mybir.dt.bfloat16
    F32 = mybir.dt.float32
    EXP = mybir.ActivationFunctionType.Exp

    nc = bacc.Bacc(trn_type="TRN2", target_bir_lowering=False, debug=False,
                   num_devices=NCORES)

    xT_d = nc.declare_dram_parameter("xT", [D, RT], BF, isOutput=False)
    wq_d = nc.declare_dram_parameter("wq", [16, 16, 128, 128], BF, isOutput=False)
    wk_d = nc.declare_dram_parameter("wk", [16, 4, 128, 128], BF, isOutput=False)
    wv_d = nc.declare_dram_parameter("wv", [D, KD], BF, isOutput=False)
    wo_d = nc.declare_dram_parameter("wo", [D, D], BF, isOutput=False)
    crep_d = nc.declare_dram_parameter("crep", [128, RT], BF, isOutput=False)
    ssign_d = nc.declare_dram_parameter("ssign", [128, RT], BF, isOutput=False)
    mask_d = nc.declare_dram_parameter("maskm", [NBLK, 128, 512], BF,
                                       isOutput=False)
    out_d = nc.declare_dram_parameter("out", [RT, D], F32, isOutput=True)
    dbg = bool(int(os.environ.get("KDBG", "0")))
    if dbg:
        dbg_d = nc.declare_dram_parameter("dbg", [512, 1088], BF, isOutput=True)

    with tile.TileContext(nc) as tc:
        with tc.tile_pool(name="dram", bufs=1, space="DRAM") as dpool, \
             tc.tile_pool(name="const", bufs=1) as cpool, \
             tc.tile_pool(name="persist", bufs=1) as ppool, \
             tc.tile_pool(name="wstream", bufs=6) as wpool, \
             tc.tile_pool(name="work", bufs=3) as tpool, \
             tc.tile_pool(name="attn", bufs=3) as apool, \
             tc.tile_pool(name="ps", bufs=1, space="PSUM") as pspool:

            # block-major K/V exchange buffers:
            # contribK rows = (l, b, g, f, p): l=0 this core's LO block (bi),
            # l=1 HI block (bj); g=kv pair, f=fam a/b, p=hd/2.
            # contribV rows = (l, b, vrow).
            contribK = dpool.tile([2 * B * KD, 128], BF, name="contribK")
            contribV = dpool.tile([2 * B * 128, 4 * VROW], BF, name="contribV")
            gathK = dpool.tile([NCORES * 2 * B * KD, 128], BF,
                               name="gathK", addr_space="Shared")
            gathV = dpool.tile([NCORES * 2 * B * 128, 4 * VROW], BF,
                               name="gathV", addr_space="Shared")
            # rank r's LO block is block r; rank r's HI block is block 15-r.
            gKv = gathK.rearrange("(r l b g f p) c -> l b g f p r c",
                                  r=NCORES, l=2, b=B, g=4, f=2)
            gVv = gathV.rearrange("(r l b p) c -> l b p r c",
                                  r=NCORES, l=2, b=B)

            # ---- constants ----
            crep = cpool.tile([128, RT], BF, name="crep", tag="crep")
            nc.sync.dma_start(out=crep[:, :], in_=crep_d[:, :])
            ssign = cpool.tile([128, RT], BF, name="ssign", tag="ssign")
            nc.sync.dma_start(out=ssign[:, :], in_=ssign_d[:, :])
            msk = []
            for kb in range(NBLK):
                mt = cpool.tile([128, 512], BF, name=f"msk{kb}", tag=f"msk{kb}")
                nc.sync.dma_start(out=mt[:, :], in_=mask_d[kb, :, :])
                msk.append(mt)

            # ---- xT resident ----
            xt = []
            for k in range(16):
                t_ = ppool.tile([128, RT], BF, name=f"xt{k}", tag=f"xt{k}")
                nc.sync.dma_start(out=t_[:, :], in_=xT_d[k * 128:(k + 1) * 128, :])
                xt.append(t_)

            # ---- K projection + RoPE -> contribK ----
            for g in range(4):
                ps = pspool.tile([128, RT], F32, name=f"psk{g}", tag=f"pv{g % 4}")
                for kt in range(16):
                    wkt = wpool.tile([128, 128], BF, name="wkt", tag="wk")
                    (nc.sync if kt % 2 == 0 else nc.gpsimd).dma_start(
                        out=wkt[:, :], in_=wk_d[kt, g, :, :])
                    nc.tensor.matmul(ps[:, :], lhsT=wkt[:, :], rhs=xt[kt][:, :],
                                     start=(kt == 0), stop=(kt == 15))
                kraw = tpool.tile([128, RT], BF, name="kraw", tag="kraw")
                nc.vector.tensor_copy(out=kraw[:, :], in_=ps[:, :])
                kt_t = tpool.tile([128, RT], BF, name=f"kT{g}", tag="kTout")
                rot = tpool.tile([128, RT], BF, name="rot", tag="rot")
                for (db, sb) in ((0, 32), (32, 0), (64, 96), (96, 64)):
                    nc.gpsimd.dma_start(out=rot[db:db + 32, :],
                                        in_=kraw[sb:sb + 32, :])
                t2 = tpool.tile([128, RT], BF, name="ropea", tag="ropea")
                t3 = tpool.tile([128, RT], BF, name="ropeb", tag="ropeb")
                nc.vector.tensor_mul(t2[:, :], kraw[:, :], crep[:, :])
                nc.vector.tensor_mul(t3[:, :], rot[:, :], ssign[:, :])
                nc.vector.tensor_add(kt_t[:, :], t2[:, :], t3[:, :])
                # kt_t cols per batch are [HI(128) | LO(128)] -> l=1, l=0
                dstv = contribK.rearrange("(l b g f p) c -> g l f p b c",
                                          l=2, b=B, g=4, f=2)
                kv = kt_t.rearrange("p (b l c) -> p b l c", b=B, l=2)
                for f in range(2):
                    for l_src, l_dst in ((0, 1), (1, 0)):
                        nc.sync.dma_start(
                            out=dstv[g, l_dst, f, :, :, :],
                            in_=kv[f * 64:(f + 1) * 64, :, l_src, :])

            # ---- V projection -> contribV (with ones columns) ----
            for r in range(4):
                # row quarter r = (b, pos): 0=(b0,HI) 1=(b0,LO) 2=(b1,HI) 3=(b1,LO)
                b_, pos = divmod(r, 2)
                l = 1 - pos  # HI -> l=1, LO -> l=0
                ps = pspool.tile([128, KD], F32, name=f"psv{r}", tag=f"pv{r % 4}")
                for kt in range(16):
                    wvt = wpool.tile([128, KD], BF, name="wvt", tag="wv")
                    (nc.sync if kt % 2 == 0 else nc.gpsimd).dma_start(
                        out=wvt[:, :], in_=wv_d[kt * 128:(kt + 1) * 128, :])
                    nc.tensor.matmul(ps[:, :], lhsT=xt[kt][:, r * 128:(r + 1) * 128],
                                     rhs=wvt[:, :], start=(kt == 0), stop=(kt == 15))
                vsb = tpool.tile([128, 4 * VROW], BF, name="vsb", tag="vsb")
                vdst = vsb.rearrange("p (g t u) -> p g t u", g=4, t=2, u=VROW // 2)
                vsrc = ps.rearrange("p (g t u) -> p g t u", g=4, t=2, u=HD)
                nc.scalar.copy(out=vdst[:, :, :, 0:HD], in_=vsrc[:, :, :, :])
                nc.gpsimd.memset(vdst[:, :, :, HD:HD + 1], 1.0)
                nc.sync.dma_start(
                    out=contribV[(l * B + b_) * 128:(l * B + b_ + 1) * 128, :],
                    in_=vsb[:, :])

            # ---- AllGather K/V ----
            nc.gpsimd.collective_compute(
                "AllGather", mybir.AluOpType.bypass,
                replica_groups=[list(range(NCORES))],
                ins=[contribK[:, :].opt()], outs=[gathK[:, :].opt()],
            )
            nc.gpsimd.collective_compute(
                "AllGather", mybir.AluOpType.bypass,
                replica_groups=[list(range(NCORES))],
                ins=[contribV[:, :].opt()], outs=[gathV[:, :].opt()],
            )

            # ---- Q projection + RoPE (overlaps the AllGather) ----
            # qpa/qpb[gg][p]: [64, 1024], per-batch cols
            # [h(2p):HI | h(2p+1):HI | h(2p):LO | h(2p+1):LO]  (128 each);
            # a/b = first/second head of the GQA pair (kv 2gg / 2gg+1).
            qpa = [[None, None] for _ in range(4)]
            qpb = [[None, None] for _ in range(4)]
            for gg in range(4):
                for p in range(2):
                    qpa[gg][p] = ppool.tile([64, 1024], BF, name=f"qpa{gg}{p}",
                                            tag=f"qpa{gg}{p}")
                    qpb[gg][p] = ppool.tile([64, 1024], BF, name=f"qpb{gg}{p}",
                                            tag=f"qpb{gg}{p}")
            for t in range(16):
                gg, m = divmod(t, 4)
                p, hh = divmod(m, 2)
                ps = pspool.tile([128, RT], F32, name=f"psq{t}", tag=f"pv{t % 4}")
                for kt in range(16):
                    wqt = wpool.tile([128, 128], BF, name="wqt", tag="wq")
                    (nc.sync if kt % 2 == 0 else nc.gpsimd).dma_start(
                        out=wqt[:, :], in_=wq_d[kt, t, :, :])
                    nc.tensor.matmul(ps[:, :], lhsT=wqt[:, :], rhs=xt[kt][:, :],
                                     start=(kt == 0), stop=(kt == 15))
                qraw = tpool.tile([128, RT], BF, name="qraw", tag="qraw")
                nc.vector.tensor_copy(out=qraw[:, :], in_=ps[:, :])
                rot = tpool.tile([128, RT], BF, name="rot", tag="rot")
                for (db, sb) in ((0, 32), (32, 0), (64, 96), (96, 64)):
                    nc.gpsimd.dma_start(out=rot[db:db + 32, :],
                                        in_=qraw[sb:sb + 32, :])
                t2 = tpool.tile([128, RT], BF, name="ropea", tag="ropea")
                t3 = tpool.tile([128, RT], BF, name="ropeb", tag="ropeb")
                nc.vector.tensor_mul(t2[:, :], qraw[:, :], crep[:, :])
                nc.vector.tensor_mul(t3[:, :], rot[:, :], ssign[:, :])
                # src cols per batch are [HI(128) | LO(128)]; dest view drops
                # each 128-chunk at b*512 + two*256 + hh*128.
                t2v = t2.rearrange("p (b two c) -> p b two c", b=2, two=2)
                t3v = t3.rearrange("p (b two c) -> p b two c", b=2, two=2)
                for fam, qgrp in ((0, qpa), (1, qpb)):
                    pbase = fam * 64
                    dst = qgrp[gg][p].rearrange(
                        "p (b two h c) -> p b two h c", b=2, two=2, h=2)
                    for b_ in range(2):
                        nc.vector.tensor_add(
                            dst[0:64, b_, :, hh, :],
                            t2v[pbase:pbase + 64, b_, :, :],
                            t3v[pbase:pbase + 64, b_, :, :])

            # ---- attention ----
            attnT = []
            for t in range(16):
                at = ppool.tile([128, RT], BF, name=f"attnT{t}", tag=f"attnT{t}")
                attnT.append(at)

            KEYS = (("a", 0), ("a", 1), ("b", 0), ("b", 1))

            def load_bg(b, gg):
                """Fetch K/V slot data for one (b, gg): 6 strided DMAs."""
                ks = {}
                for li, lname in enumerate(("LO", "HI")):
                    for fi, fam in enumerate("ab"):
                        kt8 = apool.tile([64, 8 * 128], BF, name="kt8",
                                         tag=f"k{lname}{fam}", bufs=3)
                        (nc.sync if fi == 0 else nc.gpsimd).dma_start(
                            out=kt8.rearrange("p (r c) -> p r c", r=8),
                            in_=gKv[li, b, gg, fi, :, :, :])
                        ks[(lname, fam)] = kt8
                    vt8 = apool.tile([128, 8 * VROW], BF, name="vt8",
                                     tag=f"v{lname}", bufs=3)
                    (nc.sync if li == 0 else nc.gpsimd).dma_start(
                        out=vt8.rearrange("p (r c) -> p r c", r=8),
                        in_=gVv[li, b, :, :, VROW * gg:VROW * (gg + 1)])
                    ks[("V", lname)] = vt8
                return ks

            bgs = [(b, gg) for b in range(B) for gg in range(4)]
            ksq = {}
            ksq[bgs[0]] = load_bg(*bgs[0])
            for it, (b, gg) in enumerate(bgs):
                ks = ksq.pop((b, gg))
                if dbg and b == 0 and gg == 0:
                    nc.sync.dma_start(out=dbg_d[0:64, 0:1024],
                                      in_=ks[("HI", "a")][:, :])
                    nc.sync.dma_start(out=dbg_d[64:128, 0:1024],
                                      in_=ks[("HI", "b")][:, :])
                    nc.sync.dma_start(out=dbg_d[128:256, 0:1040],
                                      in_=ks[("V", "HI")][:, :])
                    nc.sync.dma_start(out=dbg_d[256:320, 0:1024],
                                      in_=qpa[0][0][:, :])
                if it + 1 < len(bgs):
                    ksq[bgs[it + 1]] = load_bg(*bgs[it + 1])
                pv = {}
                for i_, key in enumerate(KEYS):
                    pv[key] = pspool.tile([65, 512], F32,
                                          name=f"pvb{i_}", tag=f"pv{i_}")
                pending = []
                for kb in range(NBLK):
                    full = kb < 8
                    wide = 512 if full else int(os.environ.get("HALFW", "256"))
                    # slot kb: LO gather chunk kb, or HI chunk 15-kb
                    ch = kb if full else 15 - kb
                    lname = "LO" if full else "HI"
                    vsl = ks[("V", lname)]
                    cur = []
                    for fam, qgrp, meng in (("a", qpa[gg], nc.vector),
                                            ("b", qpb[gg], nc.vector)):
                        ksl = ks[(lname, fam)]
                        sc = pspool.tile([128, 1024], F32, name=f"sc{fam}",
                                         tag=f"sc{fam}")
                        # p-blocks at stride 512 when full, packed at stride
                        # 256 (single bank, one contiguous exp) when half.
                        pstr = wide
                        for p in range(2):
                            nc.tensor.matmul(
                                sc[:, p * pstr:p * pstr + wide],
                                lhsT=ksl[:, ch * 128:(ch + 1) * 128],
                                rhs=qgrp[p][0:64, b * 512:b * 512 + wide],
                                start=True, stop=True)
                        probs = apool.tile([128, 1024], BF, name="probs",
                                           tag=f"probs{fam}", bufs=5)
                        nw = 2 * wide
                        nc.scalar.activation(
                            out=probs[:, 0:nw], in_=sc[:, 0:nw],
                            func=EXP, scale=0.125)
                        # mask mul: full slots mask the LO chunks, half slots
                        # the HI chunks; per-core mask data zeroes overcompute.
                        moff = 256 if full else 0
                        pam = apool.tile([128, 512], BF, name="pam",
                                         tag=f"pam{fam}", bufs=5)
                        pmv = pam.rearrange("p (t c) -> p t c", t=2)
                        mv = msk[kb].rearrange("p (t c) -> p t c", t=2)
                        prv2 = probs[:, 0:2 * pstr].rearrange(
                            "p (t c) -> p t c", t=2)
                        meng.tensor_mul(pmv[:, :, :],
                                        prv2[:, :, moff:moff + 256],
                                        mv[:, :, :])
                        if dbg and b == 0 and gg == 0 and kb == 8 \
                                and fam == "a":
                            nc.sync.dma_start(out=dbg_d[320:448, 0:512],
                                              in_=probs[:, 0:512])
                            nc.sync.dma_start(out=dbg_d[448:512, 0:512],
                                              in_=pam[0:64, :])
                        cur.append((fam, probs, pam))
                    pending.append((kb, full, vsl, ch, cur))
                    if len(pending) > 3:
                        _pv_flush(nc, pv, pending.pop(0))
                for item in pending:
                    _pv_flush(nc, pv, item)
                if dbg and b == 0 and gg == 0:
                    pvd = apool.tile([65, 1024], BF, name="pvd", tag="ob")
                    nc.vector.tensor_copy(out=pvd[0:65, 0:512],
                                          in_=pv[("a", 0)][0:65, :])
                    nc.vector.tensor_copy(out=pvd[0:65, 512:1024],
                                          in_=pv[("b", 1)][0:65, :])
                    nc.sync.dma_start(out=dbg_d[256:321, 0:1024],
                                      in_=pvd[0:65, :])

                # ---- normalization ----
                sums4 = apool.tile([128, 512], F32, name="sums4",
                                   tag="sums4", bufs=2)
                for i_, key in enumerate(KEYS):
                    nc.vector.tensor_copy(out=sums4[32 * i_:32 * i_ + 1, :],
                                          in_=pv[key][64:65, :])
                rec4 = apool.tile([128, 512], F32, name="rec4",
                                  tag="rec4", bufs=2)
                nc.vector.reciprocal(out=rec4[:, :], in_=sums4[:, :])
                for i_, (fam, p) in enumerate(KEYS):
                    rec2 = apool.tile([1, 512], F32, name="rec2",
                                      tag="rec2", bufs=2)
                    # partition_broadcast reads physical partition 0 of its
                    # source tile, so stage the row into a row-0 tile first.
                    nc.vector.tensor_copy(out=rec2[0:1, :],
                                          in_=rec4[32 * i_:32 * i_ + 1, :])
                    rep = apool.tile([128, 512], F32, name="repbc",
                                     tag="repbc", bufs=2)
                    nc.gpsimd.partition_broadcast(rep[:, :], rec2[0:1, :])
                    pvv = pv[(fam, p)].rearrange(
                        "p (two h c) -> p two h c", two=2, h=2)
                    repv = rep.rearrange(
                        "p (two h c) -> p two h c", two=2, h=2)
                    pbase = 0 if fam == "a" else 64
                    for hh in range(2):
                        t = 4 * gg + 2 * p + hh
                        # pv cols [h:HI | h:LO] (stride 256) -> attnT cols
                        # [HI | LO] contiguous at b*RPB
                        nc.vector.tensor_mul(
                            attnT[t][pbase:pbase + 64,
                                     b * RPB:b * RPB + 256].rearrange(
                                "p (two c) -> p two c", two=2),
                            pvv[0:64, :, hh, :],
                            repv[pbase:pbase + 64, :, hh, :])

            # ---- output projection ----
            for dc in range(4):
                po = [pspool.tile([128, 512], F32, name=f"po{rt}", tag=f"pv{rt}")
                      for rt in range(4)]
                for t in range(16):
                    wot = wpool.tile([128, 512], BF, name="wot", tag="wo")
                    (nc.sync if t % 2 == 0 else nc.gpsimd).dma_start(
                        out=wot[:, :],
                        in_=wo_d[t * 128:(t + 1) * 128, dc * 512:(dc + 1) * 512])
                    for rt in range(4):
                        nc.tensor.matmul(po[rt][:, :],
                                         lhsT=attnT[t][:, rt * 128:(rt + 1) * 128],
                                         rhs=wot[:, :],
                                         start=(t == 0), stop=(t == 15))
                for rt in range(4):
                    ob = apool.tile([128, 512], F32, name="ob", tag="ob")
                    nc.vector.tensor_copy(out=ob[:, :], in_=po[rt][:, :])
                    nc.sync.dma_start(
                        out=out_d[rt * 128:(rt + 1) * 128,
                                  dc * 512:(dc + 1) * 512],
                        in_=ob[:, :])

    nc.compile()
    return nc


def _pv_flush(nc, pv, item):
    """Issue the (delayed) PV matmuls for one key block."""
    kb, full, vsl, ch, cur = item
    for fam, probs, pam in cur:
        vcol = ch * VROW + (0 if fam == "a" else 65)
        for p in range(2):
            dst = pv[(fam, p)]
            if full:
                # unmasked HI cols + masked LO cols
                # NOTE: start=True zeroes the whole PSUM bank, so only the
                # first matmul touching the bank may set it.
                nc.tensor.matmul(
                    dst[0:65, 0:256],
                    lhsT=vsl[:, vcol:vcol + 65],
                    rhs=probs[:, p * 512:p * 512 + 256],
                    start=(kb == 0), stop=False,
                    skip_group_check=(kb > 0))
                nc.tensor.matmul(
                    dst[0:65, 256:512],
                    lhsT=vsl[:, vcol:vcol + 65],
                    rhs=pam[:, p * 256:(p + 1) * 256],
                    start=False, stop=(kb == 7),
                    skip_group_check=True)
            else:
                nc.tensor.matmul(
                    dst[0:65, 0:256],
                    lhsT=vsl[:, vcol:vcol + 65],
                    rhs=pam[:, p * 256:(p + 1) * 256],
                    start=False, stop=(kb == 15),
                    skip_group_check=True)


# --------------------------------------------------------------------------
# host-side sharding / layout prep
# --------------------------------------------------------------------------

def _prep_shared(wq, wk, wv, wo):
    qcol = np.zeros(D, np.int64)
    worow = np.zeros(D, np.int64)
    for t in range(16):
        ha, hb = _heads_of_tile(t)
        for half, h in enumerate((ha, hb)):
            base = t * 128 + half * 64
            qcol[base:base + 32] = h * 64 + np.arange(0, 64, 2)
            qcol[base + 32:base + 64] = h * 64 + np.arange(1, 64, 2)
            worow[base:base + 64] = h * 64 + np.arange(64)
    kcol = np.zeros(KD, np.int64)
    for g in range(NKV):
        base = g * 64
        kcol[base:base + 32] = g * 64 + np.arange(0, 64, 2)
        kcol[base + 32:base + 64] = g * 64 + np.arange(1, 64, 2)

    wq_t = wq[:, qcol].reshape(16, 128, 16, 128).transpose(0, 2, 1, 3)
    wq_t = np.ascontiguousarray(wq_t).astype(BF16)
    wk_t = wk[:, kcol].reshape(16, 128, 4, 128).transpose(0, 2, 1, 3)
    wk_t = np.ascontiguousarray(wk_t).astype(BF16)
    wv_c = np.ascontiguousarray(wv).astype(BF16)
    wo_c = np.ascontiguousarray(wo[worow, :]).astype(BF16)
    return wq_t, wk_t, wv_c, wo_c


def _prep_core(i, x, freqs_cos, freqs_sin, mask):
    bi, bj = _core_blocks(i)
    # row order per batch: [HI (bj block) | LO (bi block)]
    rows = np.concatenate([np.arange(bj * BLK, (bj + 1) * BLK),
                           np.arange(bi * BLK, (bi + 1) * BLK)])
    xs = np.concatenate([x[0, rows, :], x[1, rows, :]], axis=0)       # [512, D]
    xT = np.ascontiguousarray(xs.T).astype(BF16)                      # [D, 512]

    posf = np.concatenate([rows, rows])                               # [512]
    j = np.arange(128) % 32
    crep = freqs_cos[posf][:, j].T.astype(BF16)                       # [128, 512]
    sgn = np.where((np.arange(128) // 32) % 2 == 0, -1.0, 1.0).astype(np.float32)
    ssign = (freqs_sin[posf][:, j].T * sgn[:, None]).astype(BF16)

    # mask factors: kb<8 masks the LO (bi) rows, kb>=8 the HI (bj) rows;
    # [128 keys, 128 q].T tiled 4x wide (2 p-chunks x 2 heads)
    maskm = np.zeros((NBLK, 128, 512), np.float32)
    for kb in range(NBLK):
        blkq = bi if kb < 8 else bj
        madd = mask[blkq * BLK:(blkq + 1) * BLK, kb * BLK:(kb + 1) * BLK]
        maskm[kb] = np.tile(np.exp(madd.T), (1, 4))
    maskm = maskm.astype(BF16)
    return xT, crep, ssign, maskm


def _assemble(results):
    out = np.empty((B, S, D), np.float32)
    for i in range(NCORES):
        bi, bj = _core_blocks(i)
        r = results[i]["out"]
        out[0, bj * BLK:(bj + 1) * BLK] = r[0:128]
        out[0, bi * BLK:(bi + 1) * BLK] = r[128:256]
        out[1, bj * BLK:(bj + 1) * BLK] = r[256:384]
        out[1, bi * BLK:(bi + 1) * BLK] = r[384:512]
    return out


LAST_RUN_INFO = {}


def kernel(x, freqs_cos, freqs_sin, mask, wq, wk, wv, wo, start_pos=0):
    from concourse.bass_utils import run_bass_kernel_spmd

    x = np.asarray(x, dtype=np.float32)
    freqs_cos = np.asarray(freqs_cos, dtype=np.float32)
    freqs_sin = np.asarray(freqs_sin, dtype=np.float32)
    mask = np.asarray(mask, dtype=np.float32)
    wq = np.asarray(wq, dtype=np.float32)
    wk = np.asarray(wk, dtype=np.float32)
    wv = np.asarray(wv, dtype=np.float32)
    wo = np.asarray(wo, dtype=np.float32)

    wq_t, wk_t, wv_c, wo_c = _prep_shared(wq, wk, wv, wo)
    in_maps = []
    for i in range(NCORES):
        xT, crep, ssign, maskm = _prep_core(i, x, freqs_cos, freqs_sin, mask)
        in_maps.append({
            "xT": xT, "wq": wq_t, "wk": wk_t, "wv": wv_c, "wo": wo_c,
            "crep": crep, "ssign": ssign, "maskm": maskm,
        })

    nc = _build_nc()

    trace = bool(int(os.environ.get("KERNEL_TRACE", "0")))
    kwargs = {}
    if trace:
        _install_ntff_hook()
        import concourse.bass_utils as bass_utils
        bass_utils.upload_artifacts = lambda tmpdir: tmpdir
        import tempfile
        tmpdir = tempfile.mkdtemp(prefix="attn_trace_")
        kwargs = {"trace": True, "tmpdir": tmpdir}

    res = run_bass_kernel_spmd(nc, in_maps, core_ids=list(range(NCORES)),
                               **kwargs)
    LAST_RUN_INFO.clear()
    LAST_RUN_INFO.update({
        "exec_time_ns": res.exec_time_ns,
        "tmpdir": kwargs.get("tmpdir"),
        "res": res,
        "dbg": [r.get("dbg") for r in res.results],
    })
    return _assemble(res.results)


def _install_ntff_hook():
    if "antenv.axon_hooks" not in sys.modules:
        import antenv

        mod = types.ModuleType("antenv.axon_hooks")
        mod._hook = None
        mod.set_axon_ntff_profile_hook = lambda h: setattr(mod, "_hook", h)
        mod.get_axon_ntff_profile_hook = lambda: mod._hook
        sys.modules["antenv.axon_hooks"] = mod
        antenv.axon_hooks = mod
    from trn_agent_boot.trn_boot import _ntff_profile_via_ctypes
    from antenv.axon_hooks import set_axon_ntff_profile_hook as _set

    _set(_ntff_profile_via_ctypes("/opt/axon/libaxon_pjrt.so"))


# revision 24
# speedup vs baseline: 1.3753x; 1.3753x over previous
"""Distributed GQA attention kernel for 8 TRN2 NeuronCores.

Problem: B=2, S=2048, D=2048, 32 q-heads / 8 kv-heads, hd=64, causal + RoPE.

Strategy (sequence-sharded "context parallel", single SPMD program):
  - Each core owns 2 zigzag row-blocks per batch (blocks bj=15-i and bi=i of
    16, stored [bj | bi]), 512 rows total. It computes Q for all 32 heads on
    its rows, K/V for all 8 kv-heads on its rows, applies RoPE, then
    AllGathers K/V in block-major layout (~1MB/rank).
  - Attention runs fully "transposed": projections produce qT/kT (head-dim on
    partitions) directly from x^T (host-pretransposed), scoresT = kT_tile.T @
    qT come out with keys on partitions, probsT feeds P@V as the moving
    operand with V as the stationary operand, and the PV output outT
    [hd, rows] is exactly the lhsT layout the output projection needs.
  - Uniform causal-skip width profile: q cols per batch are laid out
    [h0:HI | h1:HI | h0:LO | h1:LO] (HI = bj block rows, LO = bi rows).
    Key blocks kb=0..7 run 512-wide (every core needs its LO rows there and
    all HI rows attend them unmasked); kb=8..15 run 256-wide (HI only).
    This covers every core's causal needs with one instruction stream; the
    per-core mask *data* (multiplicative exp(mask) factors) zeroes the
    overcomputed region. 6144 score-cycles/combo vs 8192 unskipped.
  - Scores for the (a,p0)/(a,p1) head pairs land in one 2-bank PSUM tile
    [128,1024] (likewise the b pairs) so one ACT instruction exps both.
  - Softmax without max-subtraction: probs = exp(s/8). Full slots multiply
    masks only into the LO half (HI is always below-diagonal there); half
    slots multiply the HI cols. A-group muls run on Vector, B-group on
    GpSimd. The denominator comes free from a ones-column appended to V
    (M=65 PV matmuls); normalization multiplies the attention output.
  - Matmuls run in bf16 (1 cycle/row); psums/softmax stay fp32.

kernel(**inputs) -> np.ndarray  takes full inputs, returns full [2,2048,2048].
"""

import functools
import os
import sys
import types

import numpy as np
import ml_dtypes


BF16 = ml_dtypes.bfloat16

B, S, D = 2, 2048, 2048
NH, NKV, HD = 32, 8, 64
NREP = NH // NKV
NCORES = 8
BLK = 128
NBLK = S // BLK          # 16 blocks per batch
RPB = 2 * BLK            # rows per core per batch (2 blocks)
RT = B * RPB             # rows per core total = 512
KD = NKV * HD            # 512
VROW = 2 * HD + 2        # 130: [v_a | 1 | v_b | 1] per kv pair


def _heads_of_tile(t):
    gg, m = divmod(t, 4)
    return 8 * gg + m, 8 * gg + 4 + m


def _core_blocks(i):
    return i, NBLK - 1 - i


# --------------------------------------------------------------------------
# device graph
# --------------------------------------------------------------------------

@functools.lru_cache(maxsize=None)
def _build_nc():
    import concourse.bacc as bacc
    import concourse.mybir as mybir
    import concourse.tile as tile

    BF = mybir.dt.bfloat16
    F32 = mybir.dt.float32
    EXP = mybir.ActivationFunctionType.Exp

    nc = bacc.Bacc(trn_type="TRN2", target_bir_lowering=False, debug=False,
                   num_devices=NCORES)

    xT_d = nc.declare_dram_parameter("xT", [D, RT], BF, isOutput=False)
    wq_d = nc.declare_dram_parameter("wq", [16, 16, 128, 128], BF, isOutput=False)
    wk_d = nc.declare_dram_parameter("wk", [16, 4, 128, 128], BF, isOutput=False)
    wv_d = nc.declare_dram_parameter("wv", [D, KD], BF, isOutput=False)
    wo_d = nc.declare_dram_parameter("wo", [D, D], BF, isOutput=False)
    crep_d = nc.declare_dram_parameter("crep", [128, RT], BF, isOutput=False)
    ssign_d = nc.declare_dram_parameter("ssign", [128, RT], BF, isOutput=False)
    mask_d = nc.declare_dram_parameter("maskm", [NBLK, 128, 512], BF,
                                       isOutput=False)
    out_d = nc.declare_dram_parameter("out", [RT, D], F32, isOutput=True)
    dbg = bool(int(os.environ.get("KDBG", "0")))
    if dbg:
        dbg_d = nc.declare_dram_parameter("dbg", [512, 1088], BF, isOutput=True)

    with tile.TileContext(nc) as tc:
        with tc.tile_pool(name="dram", bufs=1, space="DRAM") as dpool, \
             tc.tile_pool(name="const", bufs=1) as cpool, \
             tc.tile_pool(name="persist", bufs=1) as ppool, \
             tc.tile_pool(name="wstream", bufs=6) as wpool, \
             tc.tile_pool(name="work", bufs=3) as tpool, \
             tc.tile_pool(name="attn", bufs=3) as apool, \
             tc.tile_pool(name="ps", bufs=1, space="PSUM") as pspool:

            # block-major K/V exchange buffers:
            # contribK rows = (l, b, g, f, p): l=0 this core's LO block (bi),
            # l=1 HI block (bj); g=kv pair, f=fam a/b, p=hd/2.
            # contribV rows = (l, b, vrow).
            contribK = dpool.tile([2 * B * KD, 128], BF, name="contribK")
            contribV = dpool.tile([2 * B * 128, 4 * VROW], BF, name="contribV")
            gathK = dpool.tile([NCORES * 2 * B * KD, 128], BF,
                               name="gathK", addr_space="Shared")
            gathV = dpool.tile([NCORES * 2 * B * 128, 4 * VROW], BF,
                               name="gathV", addr_space="Shared")
            # rank r's LO block is block r; rank r's HI block is block 15-r.
            gKv = gathK.rearrange("(r l b g f p) c -> l b g f p r c",
                                  r=NCORES, l=2, b=B, g=4, f=2)
            gVv = gathV.rearrange("(r l b p) c -> l b p r c",
                                  r=NCORES, l=2, b=B)

            # ---- constants ----
            crep = cpool.tile([128, RT], BF, name="crep", tag="crep")
            nc.sync.dma_start(out=crep[:, :], in_=crep_d[:, :])
            ssign = cpool.tile([128, RT], BF, name="ssign", tag="ssign")
            nc.sync.dma_start(out=ssign[:, :], in_=ssign_d[:, :])
            msk = []
            for kb in range(NBLK):
                mt = cpool.tile([128, 512], BF, name=f"msk{kb}", tag=f"msk{kb}")
                nc.sync.dma_start(out=mt[:, :], in_=mask_d[kb, :, :])
                msk.append(mt)

            # ---- xT resident ----
            xt = []
            for k in range(16):
                t_ = ppool.tile([128, RT], BF, name=f"xt{k}", tag=f"xt{k}")
                nc.sync.dma_start(out=t_[:, :], in_=xT_d[k * 128:(k + 1) * 128, :])
                xt.append(t_)

            # ---- K projection + RoPE -> contribK ----
            wkv = wk_d.rearrange("kt g p c -> g p kt c")
            for g in range(4):
                ps = pspool.tile([128, RT], F32, name=f"psk{g}", tag=f"pv{g % 4}")
                wkt = wpool.tile([128, 2048], BF, name="wkt", tag="wo", bufs=4)
                (nc.sync if g % 2 == 0 else nc.gpsimd).dma_start(
                    out=wkt.rearrange("p (kt c) -> p kt c", kt=16),
                    in_=wkv[g, :, :, :])
                for kt in range(16):
                    nc.tensor.matmul(ps[:, :], lhsT=wkt[:, kt * 128:(kt + 1) * 128],
                                     rhs=xt[kt][:, :],
                                     start=(kt == 0), stop=(kt == 15))
                kraw = tpool.tile([128, RT], BF, name="kraw", tag="kraw")
                nc.vector.tensor_copy(out=kraw[:, :], in_=ps[:, :])
                kt_t = tpool.tile([128, RT], BF, name=f"kT{g}", tag="kTout")
                rot = tpool.tile([128, RT], BF, name="rot", tag="rot")
                for (db, sb) in ((0, 32), (32, 0), (64, 96), (96, 64)):
                    nc.gpsimd.dma_start(out=rot[db:db + 32, :],
                                        in_=kraw[sb:sb + 32, :])
                t2 = tpool.tile([128, RT], BF, name="ropea", tag="ropea")
                t3 = tpool.tile([128, RT], BF, name="ropeb", tag="ropeb")
                nc.vector.tensor_mul(t2[:, :], kraw[:, :], crep[:, :])
                nc.vector.tensor_mul(t3[:, :], rot[:, :], ssign[:, :])
                nc.vector.tensor_add(kt_t[:, :], t2[:, :], t3[:, :])
                # kt_t cols per batch are [HI(128) | LO(128)] -> l=1, l=0
                dstv = contribK.rearrange("(l b g f p) c -> g l f p b c",
                                          l=2, b=B, g=4, f=2)
                kv = kt_t.rearrange("p (b l c) -> p b l c", b=B, l=2)
                for f in range(2):
                    for l_src, l_dst in ((0, 1), (1, 0)):
                        nc.sync.dma_start(
                            out=dstv[g, l_dst, f, :, :, :],
                            in_=kv[f * 64:(f + 1) * 64, :, l_src, :])

            # ---- AllGather K (overlaps V projection) ----
            nc.gpsimd.collective_compute(
                "AllGather", mybir.AluOpType.bypass,
                replica_groups=[list(range(NCORES))],
                ins=[contribK[:, :].opt()], outs=[gathK[:, :].opt()],
            )

            # ---- V projection -> contribV (with ones columns) ----
            # wv staged as 4 tiles of 4 kt column-stacked
            wvv = wv_d.rearrange("(kt p) c -> p kt c", kt=16)
            wvt4 = []
            for q4 in range(4):
                wvt = wpool.tile([128, 4 * KD], BF, name="wvt", tag="wo", bufs=4)
                (nc.sync if q4 % 2 == 0 else nc.gpsimd).dma_start(
                    out=wvt.rearrange("p (kt c) -> p kt c", kt=4),
                    in_=wvv[:, q4 * 4:(q4 + 1) * 4, :])
                wvt4.append(wvt)
            for r in range(4):
                # row quarter r = (b, pos): 0=(b0,HI) 1=(b0,LO) 2=(b1,HI) 3=(b1,LO)
                b_, pos = divmod(r, 2)
                l = 1 - pos  # HI -> l=1, LO -> l=0
                ps = pspool.tile([128, KD], F32, name=f"psv{r}", tag=f"pv{r % 4}")
                for kt in range(16):
                    wvt = wvt4[kt // 4][:, (kt % 4) * KD:(kt % 4 + 1) * KD]
                    nc.tensor.matmul(ps[:, :], lhsT=xt[kt][:, r * 128:(r + 1) * 128],
                                     rhs=wvt, start=(kt == 0), stop=(kt == 15))
                vsb = tpool.tile([128, 4 * VROW], BF, name="vsb", tag="vsb")
                vdst = vsb.rearrange("p (g t u) -> p g t u", g=4, t=2, u=VROW // 2)
                vsrc = ps.rearrange("p (g t u) -> p g t u", g=4, t=2, u=HD)
                nc.scalar.copy(out=vdst[:, :, :, 0:HD], in_=vsrc[:, :, :, :])
                nc.gpsimd.memset(vdst[:, :, :, HD:HD + 1], 1.0)
                nc.sync.dma_start(
                    out=contribV[(l * B + b_) * 128:(l * B + b_ + 1) * 128, :],
                    in_=vsb[:, :])

            # ---- AllGather V ----
            nc.gpsimd.collective_compute(
                "AllGather", mybir.AluOpType.bypass,
                replica_groups=[list(range(NCORES))],
                ins=[contribV[:, :].opt()], outs=[gathV[:, :].opt()],
            )

            # ---- Q projection + RoPE (overlaps the AllGather) ----
            # qpa/qpb[gg][p]: [64, 1024], per-batch cols
            # [h(2p):HI | h(2p+1):HI | h(2p):LO | h(2p+1):LO]  (128 each);
            # a/b = first/second head of the GQA pair (kv 2gg / 2gg+1).
            qpa = [[None, None] for _ in range(4)]
            qpb = [[None, None] for _ in range(4)]
            for gg in range(4):
                for p in range(2):
                    qpa[gg][p] = ppool.tile([64, 1024], BF, name=f"qpa{gg}{p}",
                                            tag=f"qpa{gg}{p}")
                    qpb[gg][p] = ppool.tile([64, 1024], BF, name=f"qpb{gg}{p}",
                                            tag=f"qpb{gg}{p}")
            wqv = wq_d.rearrange("kt t p c -> t p kt c")
            for t in range(16):
                gg, m = divmod(t, 4)
                p, hh = divmod(m, 2)
                ps = pspool.tile([128, RT], F32, name=f"psq{t}", tag=f"pv{t % 4}")
                wq4 = []
                for q4 in range(4):
                    wqt = wpool.tile([128, 512], BF, name="wqt", tag="wq",
                                     bufs=6)
                    (nc.sync if q4 % 2 == 0 else nc.gpsimd).dma_start(
                        out=wqt.rearrange("p (kt c) -> p kt c", kt=4),
                        in_=wqv[t, :, q4 * 4:(q4 + 1) * 4, :])
                    wq4.append(wqt)
                for kt in range(16):
                    nc.tensor.matmul(
                        ps[:, :],
                        lhsT=wq4[kt // 4][:, (kt % 4) * 128:(kt % 4 + 1) * 128],
                        rhs=xt[kt][:, :],
                        start=(kt == 0), stop=(kt == 15))
                qraw = tpool.tile([128, RT], BF, name="qraw", tag="qraw")
                nc.vector.tensor_copy(out=qraw[:, :], in_=ps[:, :])
                rot = tpool.tile([128, RT], BF, name="rot", tag="rot")
                for (db, sb) in ((0, 32), (32, 0), (64, 96), (96, 64)):
                    nc.gpsimd.dma_start(out=rot[db:db + 32, :],
                                        in_=qraw[sb:sb + 32, :])
                t2 = tpool.tile([128, RT], BF, name="ropea", tag="ropea")
                t3 = tpool.tile([128, RT], BF, name="ropeb", tag="ropeb")
                nc.vector.tensor_mul(t2[:, :], qraw[:, :], crep[:, :])
                nc.vector.tensor_mul(t3[:, :], rot[:, :], ssign[:, :])
                # src cols per batch are [HI(128) | LO(128)]; dest view drops
                # each 128-chunk at b*512 + two*256 + hh*128.
                t2v = t2.rearrange("p (b two c) -> p b two c", b=2, two=2)
                t3v = t3.rearrange("p (b two c) -> p b two c", b=2, two=2)
                for fam, qgrp in ((0, qpa), (1, qpb)):
                    pbase = fam * 64
                    dst = qgrp[gg][p].rearrange(
                        "p (b two h c) -> p b two h c", b=2, two=2, h=2)
                    for b_ in range(2):
                        nc.vector.tensor_add(
                            dst[0:64, b_, :, hh, :],
                            t2v[pbase:pbase + 64, b_, :, :],
                            t3v[pbase:pbase + 64, b_, :, :])

            # ---- attention ----
            attnT = []
            for t in range(16):
                at = ppool.tile([128, RT], BF, name=f"attnT{t}", tag=f"attnT{t}")
                attnT.append(at)

            KEYS = (("a", 0), ("a", 1), ("b", 0), ("b", 1))

            def load_bg(b, gg):
                """Fetch K/V slot data for one (b, gg): 6 strided DMAs."""
                ks = {}
                for li, lname in enumerate(("LO", "HI")):
                    for fi, fam in enumerate("ab"):
                        kt8 = apool.tile([64, 8 * 128], BF, name="kt8",
                                         tag=f"k{lname}{fam}", bufs=2)
                        (nc.sync if fi == 0 else nc.gpsimd).dma_start(
                            out=kt8.rearrange("p (r c) -> p r c", r=8),
                            in_=gKv[li, b, gg, fi, :, :, :])
                        ks[(lname, fam)] = kt8
                    vt8 = apool.tile([128, 8 * VROW], BF, name="vt8",
                                     tag=f"v{lname}", bufs=2)
                    (nc.sync if li == 0 else nc.gpsimd).dma_start(
                        out=vt8.rearrange("p (r c) -> p r c", r=8),
                        in_=gVv[li, b, :, :, VROW * gg:VROW * (gg + 1)])
                    ks[("V", lname)] = vt8
                return ks

            bgs = [(b, gg) for b in range(B) for gg in range(4)]
            ksq = {}
            ksq[bgs[0]] = load_bg(*bgs[0])
            for it, (b, gg) in enumerate(bgs):
                ks = ksq.pop((b, gg))
                if dbg and b == 0 and gg == 0:
                    nc.sync.dma_start(out=dbg_d[0:64, 0:1024],
                                      in_=ks[("HI", "a")][:, :])
                    nc.sync.dma_start(out=dbg_d[64:128, 0:1024],
                                      in_=ks[("HI", "b")][:, :])
                    nc.sync.dma_start(out=dbg_d[128:256, 0:1040],
                                      in_=ks[("V", "HI")][:, :])
                    nc.sync.dma_start(out=dbg_d[256:320, 0:1024],
                                      in_=qpa[0][0][:, :])
                if it + 1 < len(bgs):
                    ksq[bgs[it + 1]] = load_bg(*bgs[it + 1])
                pv = {}
                for i_, key in enumerate(KEYS):
                    pv[key] = pspool.tile([65, 512], F32,
                                          name=f"pvb{i_}", tag=f"pv{i_}")
                pending = []
                for kb in range(NBLK):
                    full = kb < 8
                    wide = 512 if full else int(os.environ.get("HALFW", "256"))
                    # slot kb: LO gather chunk kb, or HI chunk 15-kb
                    ch = kb if full else 15 - kb
                    lname = "LO" if full else "HI"
                    vsl = ks[("V", lname)]
                    cur = []
                    for fam, qgrp, meng in (("a", qpa[gg], nc.vector),
                                            ("b", qpb[gg], nc.vector)):
                        ksl = ks[(lname, fam)]
                        sc = pspool.tile([128, 1024], F32, name=f"sc{fam}",
                                         tag=f"sc{fam}")
                        # p-blocks at stride 512 when full, packed at stride
                        # 256 (single bank, one contiguous exp) when half.
                        pstr = wide
                        for p in range(2):
                            nc.tensor.matmul(
                                sc[:, p * pstr:p * pstr + wide],
                                lhsT=ksl[:, ch * 128:(ch + 1) * 128],
                                rhs=qgrp[p][0:64, b * 512:b * 512 + wide],
                                start=True, stop=True)
                        probs = apool.tile([128, 1024], BF, name="probs",
                                           tag=f"probs{fam}", bufs=5)
                        nw = 2 * wide
                        nc.scalar.activation(
                            out=probs[:, 0:nw], in_=sc[:, 0:nw],
                            func=EXP, scale=0.125)
                        # mask mul: full slots mask the LO chunks, half slots
                        # the HI chunks; per-core mask data zeroes overcompute.
                        moff = 256 if full else 0
                        pam = apool.tile([128, 512], BF, name="pam",
                                         tag=f"pam{fam}", bufs=5)
                        pmv = pam.rearrange("p (t c) -> p t c", t=2)
                        mv = msk[kb].rearrange("p (t c) -> p t c", t=2)
                        prv2 = probs[:, 0:2 * pstr].rearrange(
                            "p (t c) -> p t c", t=2)
                        meng.tensor_mul(pmv[:, :, :],
                                        prv2[:, :, moff:moff + 256],
                                        mv[:, :, :])
                        if dbg and b == 0 and gg == 0 and kb == 8 \
                                and fam == "a":
                            nc.sync.dma_start(out=dbg_d[320:448, 0:512],
                                              in_=probs[:, 0:512])
                            nc.sync.dma_start(out=dbg_d[448:512, 0:512],
                                              in_=pam[0:64, :])
                        cur.append((fam, probs, pam))
                    pending.append((kb, full, vsl, ch, cur))
                    if len(pending) > 3:
                        _pv_flush(nc, pv, pending.pop(0))
                for item in pending:
                    _pv_flush(nc, pv, item)
                if dbg and b == 0 and gg == 0:
                    pvd = apool.tile([65, 1024], BF, name="pvd", tag="ob")
                    nc.vector.tensor_copy(out=pvd[0:65, 0:512],
                                          in_=pv[("a", 0)][0:65, :])
                    nc.vector.tensor_copy(out=pvd[0:65, 512:1024],
                                          in_=pv[("b", 1)][0:65, :])
                    nc.sync.dma_start(out=dbg_d[256:321, 0:1024],
                                      in_=pvd[0:65, :])

                # ---- normalization ----
                sums4 = apool.tile([128, 512], F32, name="sums4",
                                   tag="sums4", bufs=2)
                for i_, key in enumerate(KEYS):
                    nc.vector.tensor_copy(out=sums4[32 * i_:32 * i_ + 1, :],
                                          in_=pv[key][64:65, :])
                rec4 = apool.tile([128, 512], F32, name="rec4",
                                  tag="rec4", bufs=2)
                nc.vector.reciprocal(out=rec4[:, :], in_=sums4[:, :])
                for i_, (fam, p) in enumerate(KEYS):
                    rec2 = apool.tile([1, 512], F32, name="rec2",
                                      tag="rec2", bufs=2)
                    # partition_broadcast reads physical partition 0 of its
                    # source tile, so stage the row into a row-0 tile first.
                    nc.vector.tensor_copy(out=rec2[0:1, :],
                                          in_=rec4[32 * i_:32 * i_ + 1, :])
                    rep = apool.tile([128, 512], F32, name="repbc",
                                     tag="repbc", bufs=2)
                    nc.gpsimd.partition_broadcast(rep[:, :], rec2[0:1, :])
                    pvv = pv[(fam, p)].rearrange(
                        "p (two h c) -> p two h c", two=2, h=2)
                    repv = rep.rearrange(
                        "p (two h c) -> p two h c", two=2, h=2)
                    pbase = 0 if fam == "a" else 64
                    for hh in range(2):
                        t = 4 * gg + 2 * p + hh
                        # pv cols [h:HI | h:LO] (stride 256) -> attnT cols
                        # [HI | LO] contiguous at b*RPB
                        nc.vector.tensor_mul(
                            attnT[t][pbase:pbase + 64,
                                     b * RPB:b * RPB + 256].rearrange(
                                "p (two c) -> p two c", two=2),
                            pvv[0:64, :, hh, :],
                            repv[pbase:pbase + 64, :, hh, :])

            # ---- output projection ----
            wov = wo_d.rearrange("(t p) c -> p t c", t=16)
            for dc in range(4):
                po = [pspool.tile([128, 512], F32, name=f"po{rt}", tag=f"pv{rt}")
                      for rt in range(4)]
                wo4 = []
                for q4 in range(4):
                    wot = wpool.tile([128, 2048], BF, name="wot", tag="wo",
                                     bufs=4)
                    (nc.sync if q4 % 2 == 0 else nc.gpsimd).dma_start(
                        out=wot.rearrange("p (t c) -> p t c", t=4),
                        in_=wov[:, q4 * 4:(q4 + 1) * 4,
                                dc * 512:(dc + 1) * 512])
                    wo4.append(wot)
                for t in range(16):
                    wot = wo4[t // 4][:, (t % 4) * 512:(t % 4 + 1) * 512]
                    for rt in range(4):
                        nc.tensor.matmul(po[rt][:, :],
                                         lhsT=attnT[t][:, rt * 128:(rt + 1) * 128],
                                         rhs=wot,
                                         start=(t == 0), stop=(t == 15))
                for rt in range(4):
                    ob = apool.tile([128, 512], F32, name="ob", tag="ob")
                    nc.vector.tensor_copy(out=ob[:, :], in_=po[rt][:, :])
                    nc.sync.dma_start(
                        out=out_d[rt * 128:(rt + 1) * 128,
                                  dc * 512:(dc + 1) * 512],
                        in_=ob[:, :])

    nc.compile()
    return nc


def _pv_flush(nc, pv, item):
    """Issue the (delayed) PV matmuls for one key block."""
    kb, full, vsl, ch, cur = item
    for fam, probs, pam in cur:
        vcol = ch * VROW + (0 if fam == "a" else 65)
        for p in range(2):
            dst = pv[(fam, p)]
            if full:
                # unmasked HI cols + masked LO cols
                # NOTE: start=True zeroes the whole PSUM bank, so only the
                # first matmul touching the bank may set it.
                nc.tensor.matmul(
                    dst[0:65, 0:256],
                    lhsT=vsl[:, vcol:vcol + 65],
                    rhs=probs[:, p * 512:p * 512 + 256],
                    start=(kb == 0), stop=False,
                    skip_group_check=(kb > 0))
                nc.tensor.matmul(
                    dst[0:65, 256:512],
                    lhsT=vsl[:, vcol:vcol + 65],
                    rhs=pam[:, p * 256:(p + 1) * 256],
                    start=False, stop=(kb == 7),
                    skip_group_check=True)
            else:
                nc.tensor.matmul(
                    dst[0:65, 0:256],
                    lhsT=vsl[:, vcol:vcol + 65],
                    rhs=pam[:, p * 256:(p + 1) * 256],
                    start=False, stop=(kb == 15),
                    skip_group_check=True)


# --------------------------------------------------------------------------
# host-side sharding / layout prep
# --------------------------------------------------------------------------

def _prep_shared(wq, wk, wv, wo):
    qcol = np.zeros(D, np.int64)
    worow = np.zeros(D, np.int64)
    for t in range(16):
        ha, hb = _heads_of_tile(t)
        for half, h in enumerate((ha, hb)):
            base = t * 128 + half * 64
            qcol[base:base + 32] = h * 64 + np.arange(0, 64, 2)
            qcol[base + 32:base + 64] = h * 64 + np.arange(1, 64, 2)
            worow[base:base + 64] = h * 64 + np.arange(64)
    kcol = np.zeros(KD, np.int64)
    for g in range(NKV):
        base = g * 64
        kcol[base:base + 32] = g * 64 + np.arange(0, 64, 2)
        kcol[base + 32:base + 64] = g * 64 + np.arange(1, 64, 2)

    wq_t = wq[:, qcol].reshape(16, 128, 16, 128).transpose(0, 2, 1, 3)
    wq_t = np.ascontiguousarray(wq_t).astype(BF16)
    wk_t = wk[:, kcol].reshape(16, 128, 4, 128).transpose(0, 2, 1, 3)
    wk_t = np.ascontiguousarray(wk_t).astype(BF16)
    wv_c = np.ascontiguousarray(wv).astype(BF16)
    wo_c = np.ascontiguousarray(wo[worow, :]).astype(BF16)
    return wq_t, wk_t, wv_c, wo_c


def _prep_core(i, x, freqs_cos, freqs_sin, mask):
    bi, bj = _core_blocks(i)
    # row order per batch: [HI (bj block) | LO (bi block)]
    rows = np.concatenate([np.arange(bj * BLK, (bj + 1) * BLK),
                           np.arange(bi * BLK, (bi + 1) * BLK)])
    xs = np.concatenate([x[0, rows, :], x[1, rows, :]], axis=0)       # [512, D]
    xT = np.ascontiguousarray(xs.T).astype(BF16)                      # [D, 512]

    posf = np.concatenate([rows, rows])                               # [512]
    j = np.arange(128) % 32
    crep = freqs_cos[posf][:, j].T.astype(BF16)                       # [128, 512]
    sgn = np.where((np.arange(128) // 32) % 2 == 0, -1.0, 1.0).astype(np.float32)
    ssign = (freqs_sin[posf][:, j].T * sgn[:, None]).astype(BF16)

    # mask factors: kb<8 masks the LO (bi) rows, kb>=8 the HI (bj) rows;
    # [128 keys, 128 q].T tiled 4x wide (2 p-chunks x 2 heads)
    maskm = np.zeros((NBLK, 128, 512), np.float32)
    for kb in range(NBLK):
        blkq = bi if kb < 8 else bj
        madd = mask[blkq * BLK:(blkq + 1) * BLK, kb * BLK:(kb + 1) * BLK]
        maskm[kb] = np.tile(np.exp(madd.T), (1, 4))
    maskm = maskm.astype(BF16)
    return xT, crep, ssign, maskm


def _assemble(results):
    out = np.empty((B, S, D), np.float32)
    for i in range(NCORES):
        bi, bj = _core_blocks(i)
        r = results[i]["out"]
        out[0, bj * BLK:(bj + 1) * BLK] = r[0:128]
        out[0, bi * BLK:(bi + 1) * BLK] = r[128:256]
        out[1, bj * BLK:(bj + 1) * BLK] = r[256:384]
        out[1, bi * BLK:(bi + 1) * BLK] = r[384:512]
    return out


LAST_RUN_INFO = {}


def kernel(x, freqs_cos, freqs_sin, mask, wq, wk, wv, wo, start_pos=0):
    from concourse.bass_utils import run_bass_kernel_spmd

    x = np.asarray(x, dtype=np.float32)
    freqs_cos = np.asarray(freqs_cos, dtype=np.float32)
    freqs_sin = np.asarray(freqs_sin, dtype=np.float32)
    mask = np.asarray(mask, dtype=np.float32)
    wq = np.asarray(wq, dtype=np.float32)
    wk = np.asarray(wk, dtype=np.float32)
    wv = np.asarray(wv, dtype=np.float32)
    wo = np.asarray(wo, dtype=np.float32)

    wq_t, wk_t, wv_c, wo_c = _prep_shared(wq, wk, wv, wo)
    in_maps = []
    for i in range(NCORES):
        xT, crep, ssign, maskm = _prep_core(i, x, freqs_cos, freqs_sin, mask)
        in_maps.append({
            "xT": xT, "wq": wq_t, "wk": wk_t, "wv": wv_c, "wo": wo_c,
            "crep": crep, "ssign": ssign, "maskm": maskm,
        })

    nc = _build_nc()

    trace = bool(int(os.environ.get("KERNEL_TRACE", "0")))
    kwargs = {}
    if trace:
        _install_ntff_hook()
        import concourse.bass_utils as bass_utils
        bass_utils.upload_artifacts = lambda tmpdir: tmpdir
        import tempfile
        tmpdir = tempfile.mkdtemp(prefix="attn_trace_")
        kwargs = {"trace": True, "tmpdir": tmpdir}

    res = run_bass_kernel_spmd(nc, in_maps, core_ids=list(range(NCORES)),
                               **kwargs)
    LAST_RUN_INFO.clear()
    LAST_RUN_INFO.update({
        "exec_time_ns": res.exec_time_ns,
        "tmpdir": kwargs.get("tmpdir"),
        "res": res,
        "dbg": [r.get("dbg") for r in res.results],
    })
    return _assemble(res.results)


def _install_ntff_hook():
    if "antenv.axon_hooks" not in sys.modules:
        import antenv

        mod = types.ModuleType("antenv.axon_hooks")
        mod._hook = None
        mod.set_axon_ntff_profile_hook = lambda h: setattr(mod, "_hook", h)
        mod.get_axon_ntff_profile_hook = lambda: mod._hook
        sys.modules["antenv.axon_hooks"] = mod
        antenv.axon_hooks = mod
    from trn_agent_boot.trn_boot import _ntff_profile_via_ctypes
    from antenv.axon_hooks import set_axon_ntff_profile_hook as _set

    _set(_ntff_profile_via_ctypes("/opt/axon/libaxon_pjrt.so"))


# revision 31
# speedup vs baseline: 1.5424x; 1.1215x over previous
"""Distributed GQA attention kernel for 8 TRN2 NeuronCores.

Problem: B=2, S=2048, D=2048, 32 q-heads / 8 kv-heads, hd=64, causal + RoPE.

Strategy (sequence-sharded "context parallel", single SPMD program):
  - Each core owns 2 zigzag row-blocks per batch (blocks bj=15-i and bi=i of
    16, stored [bj | bi]), 512 rows total. It computes Q for all 32 heads on
    its rows, K/V for all 8 kv-heads on its rows, applies RoPE, then
    AllGathers K/V in block-major layout (~1MB/rank).
  - Attention runs fully "transposed": projections produce qT/kT (head-dim on
    partitions) directly from x^T (host-pretransposed), scoresT = kT_tile.T @
    qT come out with keys on partitions, probsT feeds P@V as the moving
    operand with V as the stationary operand, and the PV output outT
    [hd, rows] is exactly the lhsT layout the output projection needs.
  - Uniform causal-skip width profile: q cols per batch are laid out
    [h0:HI | h1:HI | h0:LO | h1:LO] (HI = bj block rows, LO = bi rows).
    Key blocks kb=0..7 run 512-wide (every core needs its LO rows there and
    all HI rows attend them unmasked); kb=8..15 run 256-wide (HI only).
    This covers every core's causal needs with one instruction stream; the
    per-core mask *data* (multiplicative exp(mask) factors) zeroes the
    overcomputed region. 6144 score-cycles/combo vs 8192 unskipped.
  - Scores for the (a,p0)/(a,p1) head pairs land in one 2-bank PSUM tile
    [128,1024] (likewise the b pairs) so one ACT instruction exps both.
  - Softmax without max-subtraction: probs = exp(s/8). Full slots multiply
    masks only into the LO half (HI is always below-diagonal there); half
    slots multiply the HI cols. A-group muls run on Vector, B-group on
    GpSimd. The denominator comes free from a ones-column appended to V
    (M=65 PV matmuls); normalization multiplies the attention output.
  - Matmuls run in bf16 (1 cycle/row); psums/softmax stay fp32.

kernel(**inputs) -> np.ndarray  takes full inputs, returns full [2,2048,2048].
"""

import functools
import os
import sys
import types

import numpy as np
import ml_dtypes


BF16 = ml_dtypes.bfloat16

B, S, D = 2, 2048, 2048
NH, NKV, HD = 32, 8, 64
NREP = NH // NKV
NCORES = 8
BLK = 128
NBLK = S // BLK          # 16 blocks per batch
RPB = 2 * BLK            # rows per core per batch (2 blocks)
RT = B * RPB             # rows per core total = 512
KD = NKV * HD            # 512
VROW = 2 * HD + 2        # 130: [v_a | 1 | v_b | 1] per kv pair


def _heads_of_tile(t):
    gg, m = divmod(t, 4)
    return 8 * gg + m, 8 * gg + 4 + m


def _core_blocks(i):
    return i, NBLK - 1 - i


# --------------------------------------------------------------------------
# device graph
# --------------------------------------------------------------------------

@functools.lru_cache(maxsize=None)
def _build_nc():
    import concourse.bacc as bacc
    import concourse.mybir as mybir
    import concourse.tile as tile

    BF = mybir.dt.bfloat16
    F32 = mybir.dt.float32
    EXP = mybir.ActivationFunctionType.Exp

    nc = bacc.Bacc(trn_type="TRN2", target_bir_lowering=False, debug=False,
                   num_devices=NCORES)

    xT_d = nc.declare_dram_parameter("xT", [D, RT], BF, isOutput=False)
    wq_d = nc.declare_dram_parameter("wq", [16, 16, 128, 128], BF, isOutput=False)
    wk_d = nc.declare_dram_parameter("wk", [16, 4, 128, 128], BF, isOutput=False)
    wv_d = nc.declare_dram_parameter("wv", [D, KD], BF, isOutput=False)
    wo_d = nc.declare_dram_parameter("wo", [D, D], BF, isOutput=False)
    crep_d = nc.declare_dram_parameter("crep", [128, RT], BF, isOutput=False)
    ssign_d = nc.declare_dram_parameter("ssign", [128, RT], BF, isOutput=False)
    mask_d = nc.declare_dram_parameter("maskm", [8, 128, 512], BF,
                                       isOutput=False)
    maskp_d = nc.declare_dram_parameter("maskp", [4, 128, 1024], BF,
                                        isOutput=False)
    out_d = nc.declare_dram_parameter("out", [RT, D], F32, isOutput=True)
    dbg = bool(int(os.environ.get("KDBG", "0")))
    if dbg:
        dbg_d = nc.declare_dram_parameter("dbg", [512, 1088], BF, isOutput=True)

    with tile.TileContext(nc) as tc:
        with tc.tile_pool(name="dram", bufs=1, space="DRAM") as dpool, \
             tc.tile_pool(name="const", bufs=1) as cpool, \
             tc.tile_pool(name="persist", bufs=1) as ppool, \
             tc.tile_pool(name="wstream", bufs=6) as wpool, \
             tc.tile_pool(name="work", bufs=3) as tpool, \
             tc.tile_pool(name="attn", bufs=3) as apool, \
             tc.tile_pool(name="ps", bufs=1, space="PSUM") as pspool:

            # block-major K/V exchange buffers:
            # contribK rows = (l, b, g, f, p): l=0 this core's LO block (bi),
            # l=1 HI block (bj); g=kv pair, f=fam a/b, p=hd/2.
            # contribV rows = (l, b, vrow).
            contribK = dpool.tile([2 * B * KD, 128], BF, name="contribK")
            contribV = dpool.tile([2 * B * 128, 4 * VROW], BF, name="contribV")
            gathK = dpool.tile([NCORES * 2 * B * KD, 128], BF,
                               name="gathK", addr_space="Shared")
            gathV = dpool.tile([NCORES * 2 * B * 128, 4 * VROW], BF,
                               name="gathV", addr_space="Shared")
            # rank r's LO block is block r; rank r's HI block is block 15-r.
            gKv = gathK.rearrange("(r l b g f p) c -> l b g f p r c",
                                  r=NCORES, l=2, b=B, g=4, f=2)
            gVv = gathV.rearrange("(r l b p) c -> l b p r c",
                                  r=NCORES, l=2, b=B)

            # ---- constants ----
            crep = cpool.tile([128, RT], BF, name="crep", tag="crep")
            nc.sync.dma_start(out=crep[:, :], in_=crep_d[:, :])
            ssign = cpool.tile([128, RT], BF, name="ssign", tag="ssign")
            nc.sync.dma_start(out=ssign[:, :], in_=ssign_d[:, :])
            msk = []
            for kb in range(8):
                mt = cpool.tile([128, 512], BF, name=f"msk{kb}", tag=f"msk{kb}")
                nc.sync.dma_start(out=mt[:, :], in_=mask_d[kb, :, :])
                msk.append(mt)
            mskp = []
            for pi in range(4):
                mt = cpool.tile([128, 1024], BF, name=f"mskp{pi}",
                                tag=f"mskp{pi}")
                nc.sync.dma_start(out=mt[:, :], in_=maskp_d[pi, :, :])
                mskp.append(mt)

            # ---- xT resident ----
            xt = []
            for k in range(16):
                t_ = ppool.tile([128, RT], BF, name=f"xt{k}", tag=f"xt{k}")
                nc.sync.dma_start(out=t_[:, :], in_=xT_d[k * 128:(k + 1) * 128, :])
                xt.append(t_)

            # ---- K projection + RoPE -> contribK ----
            wkv = wk_d.rearrange("kt g p c -> g p kt c")
            for g in range(4):
                ps = pspool.tile([128, RT], F32, name=f"psk{g}", tag=f"pv{g % 4}")
                wkt = wpool.tile([128, 2048], BF, name="wkt", tag="wo", bufs=4)
                (nc.sync if g % 2 == 0 else nc.gpsimd).dma_start(
                    out=wkt.rearrange("p (kt c) -> p kt c", kt=16),
                    in_=wkv[g, :, :, :])
                for kt in range(16):
                    nc.tensor.matmul(ps[:, :], lhsT=wkt[:, kt * 128:(kt + 1) * 128],
                                     rhs=xt[kt][:, :],
                                     start=(kt == 0), stop=(kt == 15))
                kraw = tpool.tile([128, RT], BF, name="kraw", tag="kraw")
                nc.vector.tensor_copy(out=kraw[:, :], in_=ps[:, :])
                kt_t = tpool.tile([128, RT], BF, name=f"kT{g}", tag="kTout")
                rot = tpool.tile([128, RT], BF, name="rot", tag="rot")
                for (db, sb) in ((0, 32), (32, 0), (64, 96), (96, 64)):
                    nc.gpsimd.dma_start(out=rot[db:db + 32, :],
                                        in_=kraw[sb:sb + 32, :])
                t2 = tpool.tile([128, RT], BF, name="ropea", tag="ropea")
                t3 = tpool.tile([128, RT], BF, name="ropeb", tag="ropeb")
                nc.vector.tensor_mul(t2[:, :], kraw[:, :], crep[:, :])
                nc.vector.tensor_mul(t3[:, :], rot[:, :], ssign[:, :])
                nc.vector.tensor_add(kt_t[:, :], t2[:, :], t3[:, :])
                # kt_t cols per batch are [HI(128) | LO(128)] -> l=1, l=0
                dstv = contribK.rearrange("(l b g f p) c -> g l f p b c",
                                          l=2, b=B, g=4, f=2)
                kv = kt_t.rearrange("p (b l c) -> p b l c", b=B, l=2)
                for f in range(2):
                    for l_src, l_dst in ((0, 1), (1, 0)):
                        nc.sync.dma_start(
                            out=dstv[g, l_dst, f, :, :, :],
                            in_=kv[f * 64:(f + 1) * 64, :, l_src, :])

            # ---- AllGather K (overlaps V projection) ----
            nc.gpsimd.collective_compute(
                "AllGather", mybir.AluOpType.bypass,
                replica_groups=[list(range(NCORES))],
                ins=[contribK[:, :].opt()], outs=[gathK[:, :].opt()],
            )

            # ---- V projection -> contribV (with ones columns) ----
            # wv staged as 4 tiles of 4 kt column-stacked
            wvv = wv_d.rearrange("(kt p) c -> p kt c", kt=16)
            wvt4 = []
            for q4 in range(4):
                wvt = wpool.tile([128, 4 * KD], BF, name="wvt", tag="wo", bufs=4)
                (nc.sync if q4 % 2 == 0 else nc.gpsimd).dma_start(
                    out=wvt.rearrange("p (kt c) -> p kt c", kt=4),
                    in_=wvv[:, q4 * 4:(q4 + 1) * 4, :])
                wvt4.append(wvt)
            for r in range(4):
                # row quarter r = (b, pos): 0=(b0,HI) 1=(b0,LO) 2=(b1,HI) 3=(b1,LO)
                b_, pos = divmod(r, 2)
                l = 1 - pos  # HI -> l=1, LO -> l=0
                ps = pspool.tile([128, KD], F32, name=f"psv{r}", tag=f"pv{r % 4}")
                for kt in range(16):
                    wvt = wvt4[kt // 4][:, (kt % 4) * KD:(kt % 4 + 1) * KD]
                    nc.tensor.matmul(ps[:, :], lhsT=xt[kt][:, r * 128:(r + 1) * 128],
                                     rhs=wvt, start=(kt == 0), stop=(kt == 15))
                vsb = tpool.tile([128, 4 * VROW], BF, name="vsb", tag="vsb")
                vdst = vsb.rearrange("p (g t u) -> p g t u", g=4, t=2, u=VROW // 2)
                vsrc = ps.rearrange("p (g t u) -> p g t u", g=4, t=2, u=HD)
                nc.scalar.copy(out=vdst[:, :, :, 0:HD], in_=vsrc[:, :, :, :])
                nc.gpsimd.memset(vdst[:, :, :, HD:HD + 1], 1.0)
                nc.sync.dma_start(
                    out=contribV[(l * B + b_) * 128:(l * B + b_ + 1) * 128, :],
                    in_=vsb[:, :])

            # ---- AllGather V ----
            nc.gpsimd.collective_compute(
                "AllGather", mybir.AluOpType.bypass,
                replica_groups=[list(range(NCORES))],
                ins=[contribV[:, :].opt()], outs=[gathV[:, :].opt()],
            )

            # ---- Q projection + RoPE (overlaps the AllGather) ----
            # qpa/qpb[gg][p]: [64, 1024], per-batch cols
            # [h(2p):HI | h(2p+1):HI | h(2p):LO | h(2p+1):LO]  (128 each);
            # a/b = first/second head of the GQA pair (kv 2gg / 2gg+1).
            qpa = [[None, None] for _ in range(4)]
            qpb = [[None, None] for _ in range(4)]
            for gg in range(4):
                for p in range(2):
                    qpa[gg][p] = ppool.tile([64, 1024], BF, name=f"qpa{gg}{p}",
                                            tag=f"qpa{gg}{p}")
                    qpb[gg][p] = ppool.tile([64, 1024], BF, name=f"qpb{gg}{p}",
                                            tag=f"qpb{gg}{p}")
            wqv = wq_d.rearrange("kt t p c -> t p kt c")
            for t in range(16):
                gg, m = divmod(t, 4)
                p, hh = divmod(m, 2)
                ps = pspool.tile([128, RT], F32, name=f"psq{t}", tag=f"pv{t % 4}")
                wq4 = []
                for q4 in range(4):
                    wqt = wpool.tile([128, 512], BF, name="wqt", tag="wq",
                                     bufs=6)
                    (nc.sync if q4 % 2 == 0 else nc.gpsimd).dma_start(
                        out=wqt.rearrange("p (kt c) -> p kt c", kt=4),
                        in_=wqv[t, :, q4 * 4:(q4 + 1) * 4, :])
                    wq4.append(wqt)
                for kt in range(16):
                    nc.tensor.matmul(
                        ps[:, :],
                        lhsT=wq4[kt // 4][:, (kt % 4) * 128:(kt % 4 + 1) * 128],
                        rhs=xt[kt][:, :],
                        start=(kt == 0), stop=(kt == 15))
                qraw = tpool.tile([128, RT], BF, name="qraw", tag="qraw")
                nc.vector.tensor_copy(out=qraw[:, :], in_=ps[:, :])
                rot = tpool.tile([128, RT], BF, name="rot", tag="rot")
                for (db, sb) in ((0, 32), (32, 0), (64, 96), (96, 64)):
                    nc.gpsimd.dma_start(out=rot[db:db + 32, :],
                                        in_=qraw[sb:sb + 32, :])
                t2 = tpool.tile([128, RT], BF, name="ropea", tag="ropea")
                t3 = tpool.tile([128, RT], BF, name="ropeb", tag="ropeb")
                nc.vector.tensor_mul(t2[:, :], qraw[:, :], crep[:, :])
                nc.vector.tensor_mul(t3[:, :], rot[:, :], ssign[:, :])
                # src cols per batch are [HI(128) | LO(128)]; dest view drops
                # each 128-chunk at b*512 + two*256 + hh*128.
                t2v = t2.rearrange("p (b two c) -> p b two c", b=2, two=2)
                t3v = t3.rearrange("p (b two c) -> p b two c", b=2, two=2)
                for fam, qgrp in ((0, qpa), (1, qpb)):
                    pbase = fam * 64
                    dst = qgrp[gg][p].rearrange(
                        "p (b two h c) -> p b two h c", b=2, two=2, h=2)
                    for b_ in range(2):
                        nc.vector.tensor_add(
                            dst[0:64, b_, :, hh, :],
                            t2v[pbase:pbase + 64, b_, :, :],
                            t3v[pbase:pbase + 64, b_, :, :])

            # ---- attention ----
            attnT = []
            for t in range(16):
                at = ppool.tile([128, RT], BF, name=f"attnT{t}", tag=f"attnT{t}")
                attnT.append(at)

            KEYS = (("a", 0), ("a", 1), ("b", 0), ("b", 1))

            def load_bg(b, gg):
                """Fetch K/V slot data for one (b, gg): 6 strided DMAs."""
                ks = {}
                for li, lname in enumerate(("LO", "HI")):
                    for fi, fam in enumerate("ab"):
                        kt8 = apool.tile([64, 8 * 128], BF, name="kt8",
                                         tag=f"k{lname}{fam}", bufs=2)
                        (nc.sync if fi == 0 else nc.gpsimd).dma_start(
                            out=kt8.rearrange("p (r c) -> p r c", r=8),
                            in_=gKv[li, b, gg, fi, :, :, :])
                        ks[(lname, fam)] = kt8
                    vt8 = apool.tile([128, 8 * VROW], BF, name="vt8",
                                     tag=f"v{lname}", bufs=2)
                    (nc.sync if li == 0 else nc.gpsimd).dma_start(
                        out=vt8.rearrange("p (r c) -> p r c", r=8),
                        in_=gVv[li, b, :, :, VROW * gg:VROW * (gg + 1)])
                    ks[("V", lname)] = vt8
                return ks

            wov = wo_d.rearrange("(t p) c -> p t c", t=16)

            def _load_wo(dc):
                w4 = []
                for q4 in range(4):
                    wot = wpool.tile([128, 2048], BF, name="wot", tag="wo",
                                     bufs=4)
                    (nc.sync if q4 % 2 == 0 else nc.gpsimd).dma_start(
                        out=wot.rearrange("p (t c) -> p t c", t=4),
                        in_=wov[:, q4 * 4:(q4 + 1) * 4,
                                dc * 512:(dc + 1) * 512])
                    w4.append(wot)
                return w4

            bgs = [(b, gg) for b in range(B) for gg in range(4)]
            ksq = {}
            ksq[bgs[0]] = load_bg(*bgs[0])
            wo_pre = {0: _load_wo(0)}
            for it, (b, gg) in enumerate(bgs):
                ks = ksq.pop((b, gg))
                if dbg and b == 0 and gg == 0:
                    nc.sync.dma_start(out=dbg_d[0:64, 0:1024],
                                      in_=ks[("HI", "a")][:, :])
                    nc.sync.dma_start(out=dbg_d[64:128, 0:1024],
                                      in_=ks[("HI", "b")][:, :])
                    nc.sync.dma_start(out=dbg_d[128:256, 0:1040],
                                      in_=ks[("V", "HI")][:, :])
                    nc.sync.dma_start(out=dbg_d[256:320, 0:1024],
                                      in_=qpa[0][0][:, :])
                if it + 1 < len(bgs):
                    ksq[bgs[it + 1]] = load_bg(*bgs[it + 1])
                pv = {}
                for i_, key in enumerate(KEYS):
                    pv[key] = pspool.tile([65, 512], F32,
                                          name=f"pvb{i_}", tag=f"pv{i_}")
                pending = []

                def slot(step):
                    """step 0..7: full slot kb=step; step 8..11: half PAIR
                    (kb0, kb1) = (8+2j, 9+2j)."""
                    if len(pending) > 3:
                        _pv_flush(nc, pv, pending.pop(0))
                    full = step < 8
                    vsl = ks[("V", "LO" if full else "HI")]
                    cur = []
                    for fam, qgrp in (("a", qpa[gg]), ("b", qpb[gg])):
                        ksl = ks[("LO" if full else "HI", fam)]
                        sc = pspool.tile([128, 1024], F32, name=f"sc{fam}",
                                         tag=f"sc{fam}")
                        if full:
                            kb = step
                            ch = kb
                            for p in range(2):
                                nc.tensor.matmul(
                                    sc[:, p * 512:(p + 1) * 512],
                                    lhsT=ksl[:, ch * 128:(ch + 1) * 128],
                                    rhs=qgrp[p][0:64, b * 512:b * 512 + 512],
                                    start=True, stop=True)
                        else:
                            # pair: [kb0: p0|p1 | kb1: p0|p1], 256 each
                            kb0 = 8 + 2 * (step - 8)
                            for j in range(2):
                                ch = 15 - (kb0 + j)
                                for p in range(2):
                                    nc.tensor.matmul(
                                        sc[:, j * 512 + p * 256:
                                           j * 512 + (p + 1) * 256],
                                        lhsT=ksl[:, ch * 128:(ch + 1) * 128],
                                        rhs=qgrp[p][0:64,
                                                    b * 512:b * 512 + 256],
                                        start=True, stop=True)
                        probs = apool.tile([128, 1024], BF, name="probs",
                                           tag=f"probs{fam}", bufs=4)
                        nc.scalar.activation(
                            out=probs[:, :], in_=sc[:, :],
                            func=EXP, scale=0.125)
                        pam = apool.tile([128, 1024], BF, name="pam",
                                         tag=f"pam{fam}", bufs=4)
                        if full:
                            # mask the LO chunks only (HI is below-diagonal)
                            pmv = pam[:, 0:512].rearrange(
                                "p (t c) -> p t c", t=2)
                            mv = msk[step].rearrange("p (t c) -> p t c", t=2)
                            prv = probs.rearrange("p (t c) -> p t c", t=2)
                            nc.vector.tensor_mul(pmv[:, :, :],
                                                 prv[:, :, 256:512],
                                                 mv[:, :, :])
                        else:
                            nc.vector.tensor_mul(pam[:, :], probs[:, :],
                                                 mskp[step - 8][:, :])
                        cur.append((fam, probs, pam))
                    pending.append((step, full, vsl, cur))

                for step in range(12):
                    slot(step)
                for item in pending:
                    _pv_flush(nc, pv, item)
                if dbg and b == 0 and gg == 0:
                    pvd = apool.tile([65, 1024], BF, name="pvd", tag="ob")
                    nc.vector.tensor_copy(out=pvd[0:65, 0:512],
                                          in_=pv[("a", 0)][0:65, :])
                    nc.vector.tensor_copy(out=pvd[0:65, 512:1024],
                                          in_=pv[("b", 1)][0:65, :])
                    nc.sync.dma_start(out=dbg_d[256:321, 0:1024],
                                      in_=pvd[0:65, :])

                # ---- normalization ----
                sums4 = apool.tile([128, 512], F32, name="sums4",
                                   tag="sums4", bufs=2)
                for i_, key in enumerate(KEYS):
                    nc.vector.tensor_copy(out=sums4[32 * i_:32 * i_ + 1, :],
                                          in_=pv[key][64:65, :])
                rec4 = apool.tile([128, 512], F32, name="rec4",
                                  tag="rec4", bufs=2)
                nc.vector.reciprocal(out=rec4[:, :], in_=sums4[:, :])
                for i_, (fam, p) in enumerate(KEYS):
                    rec2 = apool.tile([1, 512], F32, name="rec2",
                                      tag="rec2", bufs=2)
                    # partition_broadcast reads physical partition 0 of its
                    # source tile, so stage the row into a row-0 tile first.
                    nc.vector.tensor_copy(out=rec2[0:1, :],
                                          in_=rec4[32 * i_:32 * i_ + 1, :])
                    rep = apool.tile([128, 512], F32, name="repbc",
                                     tag="repbc", bufs=2)
                    nc.gpsimd.partition_broadcast(rep[:, :], rec2[0:1, :])
                    pvv = pv[(fam, p)].rearrange(
                        "p (two h c) -> p two h c", two=2, h=2)
                    repv = rep.rearrange(
                        "p (two h c) -> p two h c", two=2, h=2)
                    pbase = 0 if fam == "a" else 64
                    for hh in range(2):
                        t = 4 * gg + 2 * p + hh
                        # pv cols [h:HI | h:LO] (stride 256) -> attnT cols
                        # [HI | LO] contiguous at b*RPB
                        nc.vector.tensor_mul(
                            attnT[t][pbase:pbase + 64,
                                     b * RPB:b * RPB + 256].rearrange(
                                "p (two c) -> p two c", two=2),
                            pvv[0:64, :, hh, :],
                            repv[pbase:pbase + 64, :, hh, :])

            # ---- output projection ----
            for dc in range(4):
                po = [pspool.tile([128, 512], F32, name=f"po{rt}", tag=f"pv{rt}")
                      for rt in range(4)]
                wo4 = wo_pre.pop(dc)
                if dc + 1 < 4:
                    wo_pre[dc + 1] = _load_wo(dc + 1)
                for t in range(16):
                    wot = wo4[t // 4][:, (t % 4) * 512:(t % 4 + 1) * 512]
                    for rt in range(4):
                        nc.tensor.matmul(po[rt][:, :],
                                         lhsT=attnT[t][:, rt * 128:(rt + 1) * 128],
                                         rhs=wot,
                                         start=(t == 0), stop=(t == 15))
                for rt in range(4):
                    ob = apool.tile([128, 512], F32, name="ob", tag="ob")
                    nc.vector.tensor_copy(out=ob[:, :], in_=po[rt][:, :])
                    nc.sync.dma_start(
                        out=out_d[rt * 128:(rt + 1) * 128,
                                  dc * 512:(dc + 1) * 512],
                        in_=ob[:, :])

    nc.compile()
    return nc


def _pv_flush(nc, pv, item):
    """Issue the (delayed) PV matmuls for one slot (full kb or half pair)."""
    step, full, vsl, cur = item
    for fam, probs, pam in cur:
        fcol = 0 if fam == "a" else 65
        for p in range(2):
            dst = pv[(fam, p)]
            if full:
                kb = step
                vcol = kb * VROW + fcol
                # unmasked HI cols + masked LO cols
                # NOTE: start=True zeroes the whole PSUM bank, so only the
                # first matmul touching the bank may set it.
                nc.tensor.matmul(
                    dst[0:65, 0:256],
                    lhsT=vsl[:, vcol:vcol + 65],
                    rhs=probs[:, p * 512:p * 512 + 256],
                    start=(kb == 0), stop=False,
                    skip_group_check=(kb > 0))
                nc.tensor.matmul(
                    dst[0:65, 256:512],
                    lhsT=vsl[:, vcol:vcol + 65],
                    rhs=pam[:, p * 256:(p + 1) * 256],
                    start=False, stop=(kb == 7),
                    skip_group_check=True)
            else:
                kb0 = 8 + 2 * (step - 8)
                for j in range(2):
                    ch = 15 - (kb0 + j)
                    vcol = ch * VROW + fcol
                    nc.tensor.matmul(
                        dst[0:65, 0:256],
                        lhsT=vsl[:, vcol:vcol + 65],
                        rhs=pam[:, j * 512 + p * 256:j * 512 + (p + 1) * 256],
                        start=False, stop=(kb0 + j == 15),
                        skip_group_check=True)


# --------------------------------------------------------------------------
# host-side sharding / layout prep
# --------------------------------------------------------------------------

def _prep_shared(wq, wk, wv, wo):
    qcol = np.zeros(D, np.int64)
    worow = np.zeros(D, np.int64)
    for t in range(16):
        ha, hb = _heads_of_tile(t)
        for half, h in enumerate((ha, hb)):
            base = t * 128 + half * 64
            qcol[base:base + 32] = h * 64 + np.arange(0, 64, 2)
            qcol[base + 32:base + 64] = h * 64 + np.arange(1, 64, 2)
            worow[base:base + 64] = h * 64 + np.arange(64)
    kcol = np.zeros(KD, np.int64)
    for g in range(NKV):
        base = g * 64
        kcol[base:base + 32] = g * 64 + np.arange(0, 64, 2)
        kcol[base + 32:base + 64] = g * 64 + np.arange(1, 64, 2)

    wq_t = wq[:, qcol].reshape(16, 128, 16, 128).transpose(0, 2, 1, 3)
    wq_t = np.ascontiguousarray(wq_t).astype(BF16)
    wk_t = wk[:, kcol].reshape(16, 128, 4, 128).transpose(0, 2, 1, 3)
    wk_t = np.ascontiguousarray(wk_t).astype(BF16)
    wv_c = np.ascontiguousarray(wv).astype(BF16)
    wo_c = np.ascontiguousarray(wo[worow, :]).astype(BF16)
    return wq_t, wk_t, wv_c, wo_c


def _prep_core(i, x, freqs_cos, freqs_sin, mask):
    bi, bj = _core_blocks(i)
    # row order per batch: [HI (bj block) | LO (bi block)]
    rows = np.concatenate([np.arange(bj * BLK, (bj + 1) * BLK),
                           np.arange(bi * BLK, (bi + 1) * BLK)])
    xs = np.concatenate([x[0, rows, :], x[1, rows, :]], axis=0)       # [512, D]
    xT = np.ascontiguousarray(xs.T).astype(BF16)                      # [D, 512]

    posf = np.concatenate([rows, rows])                               # [512]
    j = np.arange(128) % 32
    crep = freqs_cos[posf][:, j].T.astype(BF16)                       # [128, 512]
    sgn = np.where((np.arange(128) // 32) % 2 == 0, -1.0, 1.0).astype(np.float32)
    ssign = (freqs_sin[posf][:, j].T * sgn[:, None]).astype(BF16)

    # mask factors: kb<8 masks the LO (bi) rows, kb>=8 the HI (bj) rows;
    # [128 keys, 128 q].T tiled 4x wide (2 p-chunks x 2 heads)
    maskm = np.zeros((NBLK, 128, 512), np.float32)
    for kb in range(NBLK):
        blkq = bi if kb < 8 else bj
        madd = mask[blkq * BLK:(blkq + 1) * BLK, kb * BLK:(kb + 1) * BLK]
        maskm[kb] = np.tile(np.exp(madd.T), (1, 4))
    maskm = maskm.astype(BF16)
    # half-region pair tiles: [kb0 | kb1] concatenated
    maskp = np.concatenate(
        [maskm[8:16:2], maskm[9:16:2]], axis=2)          # [4, 128, 1024]
    return xT, crep, ssign, maskm[:8].copy(), np.ascontiguousarray(maskp)


def _assemble(results):
    out = np.empty((B, S, D), np.float32)
    for i in range(NCORES):
        bi, bj = _core_blocks(i)
        r = results[i]["out"]
        out[0, bj * BLK:(bj + 1) * BLK] = r[0:128]
        out[0, bi * BLK:(bi + 1) * BLK] = r[128:256]
        out[1, bj * BLK:(bj + 1) * BLK] = r[256:384]
        out[1, bi * BLK:(bi + 1) * BLK] = r[384:512]
    return out


LAST_RUN_INFO = {}


def kernel(x, freqs_cos, freqs_sin, mask, wq, wk, wv, wo, start_pos=0):
    from concourse.bass_utils import run_bass_kernel_spmd

    x = np.asarray(x, dtype=np.float32)
    freqs_cos = np.asarray(freqs_cos, dtype=np.float32)
    freqs_sin = np.asarray(freqs_sin, dtype=np.float32)
    mask = np.asarray(mask, dtype=np.float32)
    wq = np.asarray(wq, dtype=np.float32)
    wk = np.asarray(wk, dtype=np.float32)
    wv = np.asarray(wv, dtype=np.float32)
    wo = np.asarray(wo, dtype=np.float32)

    wq_t, wk_t, wv_c, wo_c = _prep_shared(wq, wk, wv, wo)
    in_maps = []
    for i in range(NCORES):
        xT, crep, ssign, maskm, maskp = _prep_core(
            i, x, freqs_cos, freqs_sin, mask)
        in_maps.append({
            "xT": xT, "wq": wq_t, "wk": wk_t, "wv": wv_c, "wo": wo_c,
            "crep": crep, "ssign": ssign, "maskm": maskm, "maskp": maskp,
        })

    nc = _build_nc()

    trace = bool(int(os.environ.get("KERNEL_TRACE", "0")))
    kwargs = {}
    if trace:
        _install_ntff_hook()
        import concourse.bass_utils as bass_utils
        bass_utils.upload_artifacts = lambda tmpdir: tmpdir
        import tempfile
        tmpdir = tempfile.mkdtemp(prefix="attn_trace_")
        kwargs = {"trace": True, "tmpdir": tmpdir}

    res = run_bass_kernel_spmd(nc, in_maps, core_ids=list(range(NCORES)),
                               **kwargs)
    LAST_RUN_INFO.clear()
    LAST_RUN_INFO.update({
        "exec_time_ns": res.exec_time_ns,
        "tmpdir": kwargs.get("tmpdir"),
        "res": res,
        "dbg": [r.get("dbg") for r in res.results],
    })
    return _assemble(res.results)


def _install_ntff_hook():
    if "antenv.axon_hooks" not in sys.modules:
        import antenv

        mod = types.ModuleType("antenv.axon_hooks")
        mod._hook = None
        mod.set_axon_ntff_profile_hook = lambda h: setattr(mod, "_hook", h)
        mod.get_axon_ntff_profile_hook = lambda: mod._hook
        sys.modules["antenv.axon_hooks"] = mod
        antenv.axon_hooks = mod
    from trn_agent_boot.trn_boot import _ntff_profile_via_ctypes
    from antenv.axon_hooks import set_axon_ntff_profile_hook as _set

    _set(_ntff_profile_via_ctypes("/opt/axon/libaxon_pjrt.so"))
